# revision 44
# baseline (speedup 1.0000x reference)
"""Matrix NMS (SOLOv2 gaussian decay) on 8 TRN2 NeuronCores.

Strategy: shard the pixel (h*w=40960) contraction dim across the 8 cores.
The host pre-casts the binary masks to fp8 (exact for 0/1), so each core
DMAs only 5.24MB (vs 21MB f32) on two queues and the PE consumes it
directly with DoubleRow fp8 matmuls.  Each core computes the partial Gram
upper block-rows sequentially; block-row a drains as int16 (copies
alternating DVE/ACT) straight into the AllToAll input for shard a, split
into column halves: the left half ships as its own AllToAll while blocks
4-7 are still computing, and the right half's transfer overlaps the left
half's receive-side tree-sum and epilogue.  The epilogue is row-oriented
and column-separable: union comes from a host-precomputed area table
(areas are mask sums - host input), masked IoU d uses one
approx-reciprocal DVE pass, and the two cross-candidate reductions
(compensate_iou column max, final decay max) are PE-chunk-transposes +
DVE free-dim max combined across cores by two 4KB ReduceScatter(max)
collectives; the transposed d^2 chunks are stashed in SBUF during the
first reduction so the second needs no further PE transposes.  Every
core ends with the decayed scores for its own 128 candidates; the host
concatenates the 8 slices (gather only).
"""
import sys

import numpy as np

for _p in ("/opt/trn_rl_repo",):
    if _p not in sys.path:
        sys.path.insert(0, _p)

from concourse import bacc, bass, mybir, tile
from concourse import bass_utils

N = 1024           # candidates
HWPIX = 160 * 256  # 40960 pixels
W = 8              # cores
KC = HWPIX // W    # 5120 pixel-slice per core
KT = KC // 128     # 40 k-tiles of 128
GRP = 4            # k-tiles per resident SBUF group
RB = 128           # block-row height == shard rows
NP = KT // 2       # 20 k-tile pairs (DoubleRow)
SIGMA = 2.0

F32 = mybir.dt.float32
FP8 = mybir.dt.float8e4  # e4m3: exact for 0/1 mask values
I16 = mybir.dt.int16


def build_nc(variant="full"):
    # variant: "full" = real kernel; "nocc" = collectives replaced by local
    # DMA copies (wrong math, identical local compute/DMA — timing/sim only)
    nc = bacc.Bacc(
        "TRN2", target_bir_lowering=False, debug=False,
        num_devices=W if variant == "full" else 1,
    )

    xT = nc.dram_tensor("xT", [128, KT * N], FP8, kind="ExternalInput")
    sjsc_h = nc.dram_tensor("sjsc", [RB, N], F32, kind="ExternalInput")
    scores_h = nc.dram_tensor("scores", [1, RB], F32, kind="ExternalInput")
    ident_h = nc.dram_tensor("ident", [128, 128], F32, kind="ExternalInput")
    out_h = nc.dram_tensor("out", [1, RB], F32, kind="ExternalOutput")

    RG = [list(range(W))]

    with tile.TileContext(nc) as tc:
        with (
            tc.tile_pool(name="dram", bufs=1, space="DRAM") as dramp,
            tc.tile_pool(name="xp", bufs=1) as xp,
            tc.tile_pool(name="pg", bufs=4, space="PSUM") as pgp,
            tc.tile_pool(name="gb", bufs=4) as gbp,
            tc.tile_pool(name="a2al", bufs=1) as alp,
            tc.tile_pool(name="sc", bufs=1) as scp,
            tc.tile_pool(name="epi", bufs=1) as ep,
        ):
            # AllToAll buffers split into column halves: the left half only
            # receives writes from block-rows 0-3, so it ships while blocks
            # 4-7 are still computing/draining.
            HN = N // 2
            cc_h = [dramp.tile([W * RB, HN], I16, tag=f"cc{x}", name=f"cc{x}")
                    for x in range(2)]
            a2a_h = [dramp.tile([W * RB, HN], I16, tag=f"a2a{x}", name=f"a2a{x}")
                     for x in range(2)]
            rs1_in = dramp.tile([1, N], F32, tag="rs1_in")
            rs1_out = dramp.tile([1, RB], F32, tag="rs1_out")
            rs2_in = dramp.tile([1, N], F32, tag="rs2_in")
            rs2_out = dramp.tile([1, RB], F32, tag="rs2_out")

            # constants + epilogue inputs (off the sync queue used by x loads)
            ident = scp.tile([128, 128], F32, tag="ident")
            nc.gpsimd.dma_start(ident[:], ident_h[:])
            ones_r = scp.tile([1, 128], F32, tag="ones_r")
            nc.vector.memset(ones_r[:], 1.0)
            scores = scp.tile([1, RB], F32, tag="scores")
            nc.gpsimd.dma_start(scores[:], scores_h[:])
            # ---- phase 1: fp8 x slice straight into SBUF (10 group loads
            # on two queues so the last k-tile lands in ~half the time)
            xg = [xp.tile([128, GRP, N], FP8, tag=f"x{g}", name=f"xg{g}")
                  for g in range(KT // GRP)]
            for g in range(KT // GRP):
                eng = nc.sync if g % 2 == 0 else nc.scalar
                eng.dma_start(xg[g][:], xT[:, g * GRP * N : (g + 1) * GRP * N])
            # sjsc loads behind the even x groups on the sync queue
            sjsc = scp.tile([128, N], F32, tag="sjsc")
            nc.sync.dma_start(sjsc[:], sjsc_h[:])

            def xpair(q, c0, c1):
                t = 2 * q
                g, j = t // GRP, t % GRP
                return xg[g][:, j : j + 2, c0:c1]

            def seg_pass(a, c0, c1, dve_copy, dst):
                """One accumulation pass: block-row a x columns [c0,c1) over
                the full contraction, then drain to dst as int16.  Same
                matmul/LDWEIGHTS count as a fused pass (512-col ISA limit
                already forces one matmul per (block, k-pair, col-chunk)),
                but the PSUM closes as soon as ITS columns are done."""
                wl = c1 - c0
                pg = pgp.tile([128, wl], F32, tag="pg", name=f"pg_{a}_{c0}")
                for q in range(NP):
                    nc.tensor.matmul(
                        pg[:, 0:wl],
                        xpair(q, a * 128, (a + 1) * 128),
                        xpair(q, c0, c1),
                        start=(q == 0),
                        stop=(q == NP - 1),
                        perf_mode=mybir.MatmulPerfMode.DoubleRow,
                    )
                gb16 = gbp.tile([128, wl], I16, tag="gb16")
                if dve_copy:
                    nc.vector.tensor_copy(gb16[:], pg[:])
                else:
                    nc.scalar.activation(
                        gb16[:], pg[:], mybir.ActivationFunctionType.Copy
                    )
                eng = nc.sync if dve_copy else nc.scalar
                eng.dma_start(dst, gb16[:])

            # ---- AllToAll of the left column half; fires as soon as the
            # four left-column passes are drained (the right-column passes
            # and blocks 4-7 still compute during its transfer)
            def a2a_chunk(x):
                if variant == "full":
                    nc.gpsimd.collective_compute(
                        "AllToAll",
                        mybir.AluOpType.bypass,
                        replica_groups=RG,
                        ins=[cc_h[x][:].opt()],
                        outs=[a2a_h[x][:].opt()],
                    )
                else:
                    nc.sync.dma_start(a2a_h[x][:], cc_h[x][:])

            # ---- phase 2: left-column passes of blocks 0-3 first
            for a in range(4):
                rows = slice(a * RB, (a + 1) * RB)
                seg_pass(a, a * 128, HN, a % 2 == 0,
                         cc_h[0][rows, a * 128 : HN])
            a2a_chunk(0)
            # right-column passes of blocks 0-3, then blocks 4-7
            for a in range(4):
                rows = slice(a * RB, (a + 1) * RB)
                seg_pass(a, HN, N, a % 2 == 0, cc_h[1][rows, :])
            for a in range(4, W):
                rows = slice(a * RB, (a + 1) * RB)
                seg_pass(a, a * 128, N, a % 2 == 0,
                         cc_h[1][rows, a * 128 - HN : HN])
            a2a_chunk(1)

            # ---- local tree-sum of the 8 partials of this core's block-row
            # (per half: 4 pair-loads on alternating queues, 1KB descriptors)
            summ_h = [ep.tile([128, HN], F32, tag=f"summ{x}", name=f"summ{x}")
                      for x in range(2)]

            def tree_half(x):
                lv = []
                for h in range(4):
                    lt = alp.tile([RB, 2, HN], I16, tag=f"ld{x}{h}", name=f"ld{x}{h}")
                    src = a2a_h[x][2 * h * RB : (2 * h + 2) * RB, :].rearrange(
                        "(s p) n -> p s n", p=RB
                    )
                    eng = (nc.sync, nc.scalar,
                           nc.gpsimd if x == 1 else nc.sync, nc.scalar)[h]
                    eng.dma_start(lt[:], src)
                    lv.append(lt)
                m01 = ep.tile([RB, 2, HN], I16, tag=f"m01{x}")
                nc.vector.tensor_add(m01[:], lv[0][:], lv[1][:])
                m23 = ep.tile([RB, 2, HN], I16, tag=f"m23{x}")
                nc.vector.tensor_add(m23[:], lv[2][:], lv[3][:])
                p2 = ep.tile([RB, 2, HN], I16, tag=f"p2{x}")
                nc.vector.tensor_add(p2[:], m01[:], m23[:])
                nc.vector.tensor_add(summ_h[x][:], p2[:, 0, :], p2[:, 1, :])

            # ---- row-oriented epilogue, column-separable up to the
            # column max: the left half runs while the right A2A chunk flies
            f = ep.tile([128, N], F32, tag="f")
            fT = [ep.tile([128, 4, 128], F32, tag=f"fT{x}", name=f"fT{x}")
                  for x in range(2)]

            def col_max_half(srcx, x, name, dst, keepT=None):
                """[128, HN] -> column max row written into dst (global col
                order) via PE chunk transposes + DVE free-dim reduce.
                keepT: SBUF [128,4,128] tile to stash the transposed chunks
                for later reuse."""
                tp = pgp.tile([128, 4, 128], F32, tag="pg", name=f"{name}{x}_tp")
                for k in range(4):
                    nc.tensor.transpose(
                        tp[:, k, :], srcx[:, k * 128 : (k + 1) * 128], ident[:]
                    )
                mx4 = ep.tile([128, 4], F32, tag=f"{name}{x}_mx4")
                nc.vector.tensor_reduce(
                    mx4[:], tp[:], axis=mybir.AxisListType.X, op=mybir.AluOpType.max
                )
                m4ps = pgp.tile([4, 128], F32, tag="pg", name=f"{name}{x}_m4ps")
                nc.tensor.transpose(m4ps[:], mx4[:], ident[:])
                m4s = ep.tile([4, 128], F32, tag=f"{name}{x}_m4s")
                nc.vector.tensor_copy(m4s[:], m4ps[:])
                nc.sync.dma_start(dst, m4s[:])
                if keepT is not None:
                    nc.vector.tensor_copy(keepT[:], tp[:])

            def epi_half(x):
                cols = slice(x * HN, (x + 1) * HN)
                un = ep.tile([128, HN], F32, tag=f"un{x}", name=f"un{x}")
                nc.vector.tensor_tensor(
                    un[:], sjsc[:, cols], summ_h[x][:], op=mybir.AluOpType.subtract
                )
                nc.vector.tensor_scalar(
                    un[:], un[:], 1.0, None, op0=mybir.AluOpType.max
                )
                rec = ep.tile([128, HN], F32, tag=f"rec{x}", name=f"rec{x}")
                nc.vector.reciprocal_approx_fast(rec[:], un[:])
                dx = ep.tile([128, HN], F32, tag=f"d{x}", name=f"d{x}")
                nc.vector.tensor_mul(dx[:], summ_h[x][:], rec[:])
                # d^2 feeds both reductions (max of squares = square of
                # max for nonneg d, so rs1 on d^2 yields compensate^2)
                nc.vector.tensor_mul(f[:, cols], dx[:], dx[:])
                col_max_half(
                    f[:, cols], x, "pc", rs1_in[0:1, cols], keepT=fT[x]
                )

            tree_half(0)
            epi_half(0)
            tree_half(1)
            epi_half(1)

            if variant == "full":
                nc.gpsimd.collective_compute(
                    "ReduceScatter",
                    mybir.AluOpType.max,
                    replica_groups=RG,
                    ins=[rs1_in[:].opt()],
                    outs=[rs1_out[:].opt()],
                )
            else:
                nc.sync.dma_start(rs1_out[:], rs1_in[:, 0:RB])
            crow = ep.tile([1, RB], F32, tag="crow")
            nc.sync.dma_start(crow[:], rs1_out[:])
            # broadcast c^2 across partitions: c2sb[p, i] = c^2[i]
            cbps = pgp.tile([128, RB], F32, tag="pg", name="cbps")
            nc.tensor.matmul(cbps[:], ones_r[:], crow[:], start=True, stop=True)
            # M partial = max over rows of (d^2 - c^2), computed on the
            # stashed transposed chunks: subtract + free-dim reduce on DVE
            mx8 = ep.tile([128, W], F32, tag="mx8")
            for x in range(2):
                for k in range(4):
                    nc.vector.tensor_tensor(
                        fT[x][:, k, :], fT[x][:, k, :], cbps[:],
                        op=mybir.AluOpType.subtract,
                    )
                nc.vector.tensor_reduce(
                    mx8[:, 4 * x : 4 * x + 4], fT[x][:], axis=mybir.AxisListType.X,
                    op=mybir.AluOpType.max,
                )
            m8ps = pgp.tile([W, 128], F32, tag="pg", name="m8ps")
            nc.tensor.transpose(m8ps[:], mx8[:], ident[:])
            m8s = ep.tile([W, 128], F32, tag="m8s")
            nc.vector.tensor_copy(m8s[:], m8ps[:])
            nc.sync.dma_start(rs2_in[:], m8s[:])
            if variant == "full":
                nc.gpsimd.collective_compute(
                    "ReduceScatter",
                    mybir.AluOpType.max,
                    replica_groups=RG,
                    ins=[rs2_in[:].opt()],
                    outs=[rs2_out[:].opt()],
                )
            else:
                nc.sync.dma_start(rs2_out[:], rs2_in[:, 0:RB])
            m_red = ep.tile([1, RB], F32, tag="m_red")
            nc.sync.dma_start(m_red[:], rs2_out[:])
            # out = scores * exp(-sigma * M) for this core's 128 candidates
            coeff = ep.tile([1, RB], F32, tag="coeff")
            nc.scalar.activation(
                coeff[:], m_red[:], mybir.ActivationFunctionType.Exp, scale=-SIGMA
            )
            outsb = ep.tile([1, RB], F32, tag="outsb")
            nc.vector.tensor_mul(outsb[:], coeff[:], scores[:])
            nc.scalar.dma_start(out_h[:], outsb[:])

    nc.compile()
    return nc


_NC_CACHE = {}


def _get_nc(variant="full"):
    if variant not in _NC_CACHE:
        _NC_CACHE[variant] = build_nc(variant)
    return _NC_CACHE[variant]


def make_in_maps(seg_masks, cate_labels, cate_scores):
    import ml_dtypes

    flat = np.asarray(seg_masks, dtype=np.float32).reshape(N, -1)
    labels = np.asarray(cate_labels)
    scores = np.asarray(cate_scores, dtype=np.float32)
    areas = flat.sum(axis=1)  # exact integers in f32
    xTfull = np.ascontiguousarray(flat.T)  # (40960, 1024)
    gidx = np.arange(N)
    ident = np.eye(128, dtype=np.float32)
    in_maps = []
    for c in range(W):
        rows = slice(c * RB, (c + 1) * RB)
        gr = gidx[rows]
        valid = (gidx[None, :] > gr[:, None]) & (
            labels[None, :] == labels[rows][:, None]
        )
        # masked entries get a huge union so d = inter/union underflows to ~0
        sjsc = np.where(valid, areas[None, :] + areas[rows][:, None], 1e30)
        in_maps.append(
            {
                # partition-major: row p holds k-rows {p, 128+p, ...} of this
                # core's slice; host casts to fp8 (exact for 0/1 masks)
                "xT": np.ascontiguousarray(
                    xTfull[c * KC : (c + 1) * KC]
                    .reshape(KT, 128, N)
                    .transpose(1, 0, 2)
                ).reshape(128, KT * N).astype(ml_dtypes.float8_e4m3fn),
                "sjsc": np.ascontiguousarray(sjsc, dtype=np.float32),
                "scores": np.ascontiguousarray(scores[rows].reshape(1, RB)),
                "ident": ident,
            }
        )
    return in_maps


def run_device(in_maps, trace=False):
    nc = _get_nc()
    res = bass_utils.run_bass_kernel_spmd(
        nc, in_maps, core_ids=list(range(W)), trace=trace
    )
    return res


def kernel(seg_masks, cate_labels, cate_scores):
    in_maps = make_in_maps(seg_masks, cate_labels, cate_scores)
    res = None
    for attempt in range(3):
        try:
            res = run_device(in_maps)
            break
        except Exception:
            # transient NRT_EXEC_UNIT_UNRECOVERABLE / tunnel hiccups: the
            # device usually recovers after a short pause
            if attempt == 2:
                raise
            import time

            time.sleep(30)
    outs = [np.asarray(res.results[c]["out"]).reshape(RB) for c in range(W)]
    return np.concatenate(outs).astype(np.float32)
